# revision 2
# baseline (speedup 1.0000x reference)
"""Trainium2 Bass kernel for nn_ChannelMerger.

Computation (per batch b):
    emb   = fourier_emb(positions[b])            # [C, 288]
    scores= emb @ heads.T                        # [C, O]
    w     = softmax(scores over C)
    out[b]= w.T @ meg[b]                         # [O, T]

Sharding: data-parallel over batch B=32 across 8 cores (4 batches/core).

Device formulation (v2): the PV merge is computed TRANSPOSED,
    outT[t, o] = sum_c meg[c, t] * w[c, o]
with meg tiles as the stationary operand ([csz, 128] t-columns) and the
normalized weights W' = exp(scores)/sum as the 270-wide moving operand.
This needs ceil(C/128)=3 PE passes of 270 columns per 128 t-rows
(51840 cycles/batch) vs the [o, t] orientation's ceil(C/128)*ceil(O/128)
passes of 512 (73728 cycles/batch): the O=270 remainder row-chunk (14
rows) no longer wastes a full pass. Softmax normalization is folded into
the weights (sums are computed partition-replicated via an all-ones
stationary, reciprocal'd, and multiplied into W' once per batch), so the
PSUM eviction of each [128t, 270o] tile is a pure f32->f16 copy.

The host precomputes the fourier featurization (tiny), feeds everything
fp16, and un-transposes/casts the [b, ts, p, tile, o] staged output for
free. A burst of dummy matmuls at kernel start warms the PE_HAM clock
gate (PE idles at 1.2 GHz until it has seen a ~3.4us busy window).
"""

import math

import numpy as np

import concourse.bass as bass
import concourse.mybir as mybir
import concourse.tile as tile
from concourse import bacc

F32 = mybir.dt.float32
F16 = mybir.dt.float16  # single-pass PE matmul; fp32 is 2-pass/4x slower

B, C, T = 32, 273, 8192
O, D = 270, 288
N_CORES = 8
BPC = B // N_CORES  # batches per core
MARGIN = 0.2
N_FREQ = 12
TWO_PI = 2.0 * math.pi

TS = 4096  # T super-tile (per-DMA free size)
NT = TS // 128  # 32 t-tiles (PE stationary cols) per super-tile
NSUP = T // TS  # super-tiles per batch

C_CHUNKS = [(0, 128), (128, 128), (256, C - 256)]  # contraction over channels
K_CHUNKS = [(0, 128), (128, 128), (256, D - 256)]  # emb dim (fourier features)

WARM_MM = 12  # dummy matmuls at start: ~5us of PE busy trips HAM to 2.4 GHz

_EXP = mybir.ActivationFunctionType.Exp


def _build_module() -> bass.Bass:
    nc = bacc.Bacc()
    meg_h = nc.dram_tensor("meg", [BPC, C, T], F16, kind="ExternalInput")
    embT_h = nc.dram_tensor("embT", [BPC, D, C], F16, kind="ExternalInput")
    headsTp_h = nc.dram_tensor("headsTp", [D, O], F16, kind="ExternalInput")
    # [b, ts, partition(t%128), t-tile, o]; host reassembles to [b, o, t]
    outT_h = nc.dram_tensor("outT", [BPC, NSUP, 128, NT, O], F16, kind="ExternalOutput")

    with tile.TileContext(nc) as tc:
        with (
            tc.tile_pool(name="const", bufs=1) as const,
            tc.tile_pool(name="small", bufs=2) as small,
            tc.tile_pool(name="megp", bufs=3) as megp,
            tc.tile_pool(name="outp", bufs=2) as outp,
            tc.tile_pool(name="psum", bufs=2, space="PSUM") as psum,
        ):
            # ---- persistent constants ----
            hT = []
            for ki, (k0, ksz) in enumerate(K_CHUNKS):
                t_ = const.tile([ksz, O], F16, tag=f"hT{ki}", name=f"hT{ki}")
                nc.sync.dma_start(out=t_, in_=headsTp_h[k0 : k0 + ksz, :])
                hT.append(t_)
            ones128 = const.tile([128, 128], F16, tag="ones", name="ones128")
            nc.vector.memset(ones128, 1.0)
            warm_row = const.tile([1, 512], F16, tag="warm", name="warm_row")
            nc.vector.memset(warm_row, 0.0)

            def ps_tile():
                # one 4-bank rotating slot; all psum users share the tag
                return psum.tile([128, 4, 512], F32, tag="ps", name="ps")

            # ---- PE warmup: trip the HAM clock-gate to 8/8 before the
            # real stream starts (each dummy is a 512-cycle stream) ----
            warm_ps = ps_tile()
            for _ in range(WARM_MM):
                nc.tensor.matmul(
                    warm_ps[0:1, 0, :], ones128[0:1, 0:1], warm_row, start=True, stop=True
                )

            # ---- phase 1: softmax weights for all batches ----
            wps = []
            for b in range(BPC):
                embs = []
                for ki, (k0, ksz) in enumerate(K_CHUNKS):
                    e_ = small.tile(
                        [128, C], F16, tag=f"emb{ki}", name=f"emb{ki}", bufs=2
                    )[:ksz]
                    nc.sync.dma_start(out=e_, in_=embT_h[b, k0 : k0 + ksz, :])
                    embs.append(e_)
                expT = []
                for ci, (c0, csz) in enumerate(C_CHUNKS):
                    sc = ps_tile()[:csz, 0, 0:O]
                    for ki in range(3):
                        nc.tensor.matmul(
                            sc,
                            embs[ki][:, c0 : c0 + csz],
                            hT[ki],
                            start=(ki == 0),
                            stop=(ki == 2),
                        )
                    e_ = small.tile(
                        [128, O], F16, tag=f"expT{ci}", name=f"expT{ci}", bufs=2
                    )[:csz]
                    nc.scalar.activation(e_, sc, _EXP)
                    expT.append(e_)
                # per-partition-replicated softmax sums: all-ones stationary
                sums = ps_tile()[:, 0, 0:O]
                for ci, (c0, csz) in enumerate(C_CHUNKS):
                    nc.tensor.matmul(
                        sums,
                        ones128[:csz, :],
                        expT[ci],
                        start=(ci == 0),
                        stop=(ci == 2),
                    )
                inv = small.tile([128, O], F32, tag="inv", name="inv", bufs=2)
                nc.vector.reciprocal(inv, sums)
                wp = []
                for ci, (c0, csz) in enumerate(C_CHUNKS):
                    w_ = small.tile(
                        [128, O], F16, tag=f"wp{ci}", name=f"wp{ci}", bufs=2
                    )[:csz]
                    nc.vector.tensor_mul(w_, expT[ci], inv[:csz])
                    wp.append(w_)
                wps.append(wp)

            # ---- phase 2: transposed PV, one dense PE stream ----
            for b in range(BPC):
                wp = wps[b]
                for ts in range(NSUP):
                    t0 = ts * TS
                    megs = []
                    for ci, (c0, csz) in enumerate(C_CHUNKS):
                        m_ = megp.tile([csz, TS], F16, tag=f"meg{ci}", name=f"meg{ci}")
                        nc.sync.dma_start(
                            out=m_, in_=meg_h[b, c0 : c0 + csz, t0 : t0 + TS]
                        )
                        megs.append(m_)
                    stg = outp.tile([128, NT, O], F16, tag="stg", name="stg")
                    for g in range(NT // 4):
                        pv = ps_tile()
                        for u in range(4):
                            col0 = (g * 4 + u) * 128
                            for ci in range(3):
                                nc.tensor.matmul(
                                    pv[:, u, 0:O],
                                    megs[ci][:, col0 : col0 + 128],
                                    wp[ci],
                                    start=(ci == 0),
                                    stop=(ci == 2),
                                )
                        dst = stg[:, g * 4 : (g + 1) * 4, :]
                        src = pv[:, :, 0:O]
                        # alternate eviction engine: one alone can't keep up
                        if g % 2 == 0:
                            nc.vector.tensor_copy(dst, src)
                        else:
                            nc.scalar.copy(dst, src)
                    nc.scalar.dma_start(out=outT_h[b, ts], in_=stg)
    nc.compile()
    return nc


_MODULE_CACHE: list = []


def _get_module() -> bass.Bass:
    if not _MODULE_CACHE:
        _MODULE_CACHE.append(_build_module())
    return _MODULE_CACHE[0]


def _host_prep(meg, positions, heads):
    """Shard + lay out inputs for the 8 cores."""
    freqs = (TWO_PI / (1.0 + 2.0 * MARGIN)) * np.arange(N_FREQ, dtype=np.float64)
    pos = positions.astype(np.float64) + MARGIN
    loc = (
        pos[..., 0][..., None, None] * freqs[:, None]
        + pos[..., 1][..., None, None] * freqs[None, :]
    ).reshape(B, C, N_FREQ * N_FREQ)
    embT = np.concatenate(
        [np.cos(loc), np.sin(loc)], axis=2
    ).transpose(0, 2, 1).astype(np.float16)

    headsTp = np.ascontiguousarray(heads.T).astype(np.float16)  # [288, 270]

    in_maps = []
    for k in range(N_CORES):
        sl = slice(k * BPC, (k + 1) * BPC)
        in_maps.append(
            {
                "meg": np.ascontiguousarray(meg[sl]).astype(np.float16),
                "embT": np.ascontiguousarray(embT[sl]),
                "headsTp": headsTp,
            }
        )
    return in_maps


LAST_RESULTS = None  # BassKernelResults of the most recent kernel() call


def kernel(meg: np.ndarray, positions: np.ndarray, heads: np.ndarray) -> np.ndarray:
    global LAST_RESULTS
    from concourse.bass_utils import run_bass_kernel_spmd

    nc = _get_module()
    in_maps = _host_prep(
        np.asarray(meg, dtype=np.float32),
        np.asarray(positions, dtype=np.float32),
        np.asarray(heads, dtype=np.float32),
    )
    res = run_bass_kernel_spmd(nc, in_maps, core_ids=list(range(N_CORES)))
    LAST_RESULTS = res
    outs = []
    for r in res.results:
        a = r["outT"].astype(np.float32)  # [BPC, NSUP, 128, NT, O]
        outs.append(a.transpose(0, 4, 1, 3, 2).reshape(BPC, O, T))
    return np.concatenate(outs, axis=0)


# revision 3
# speedup vs baseline: 1.4785x; 1.4785x over previous
"""Trainium2 Bass kernel for nn_ChannelMerger.

Computation (per batch b):
    emb   = fourier_emb(positions[b])            # [C, 288]
    scores= emb @ heads.T                        # [C, O]
    w     = softmax(scores over C)
    out[b]= w.T @ meg[b]                         # [O, T]

Sharding: data-parallel over batch B=32 across 8 cores (4 batches/core).

v3 notes (trace-driven):
  - PV keeps the [o, t] orientation with 512-column moving-meg streams:
    the PE_HAM clock gate only un-throttles (1.2 -> 2.4 GHz) for streams
    with high sustained array activity; 270-column out^T streams never
    warm it (measured: whole kernel stuck at K=4/8, 291us).
  - a 12-matmul full-array warmup burst (ones[128,128] stationary,
    [128,512] moving) trips the HAM SHORT window while the fourier
    embeddings DMA in, so the softmax phase and PV run at 2.4 GHz.
  - softmax normalization is folded into the weights (partition-
    replicated sums via an all-ones stationary, reciprocal, multiply),
    so every PSUM eviction is a pure f32->f16 copy.
  - each [osz, 2048] PSUM group is evicted as two 1024-column halves on
    vector + scalar concurrently: slot-release latency halves, the
    2-deep PSUM rotation never stalls the PE (baseline's mid-kernel
    K=4/8 dips came from eviction lag).
  - O-chunk loop is inside the half-supertile loop so the low-activity
    osz=14 passes never form a >3.4us stint (HAM MID re-throttle).
"""

import math

import numpy as np

import concourse.bass as bass
import concourse.mybir as mybir
import concourse.tile as tile
from concourse import bacc

F32 = mybir.dt.float32
F16 = mybir.dt.float16  # single-pass PE matmul; fp32 is 2-pass/4x slower

B, C, T = 32, 273, 8192
O, D = 270, 288
N_CORES = 8
BPC = B // N_CORES  # batches per core
MARGIN = 0.2
N_FREQ = 12
TWO_PI = 2.0 * math.pi

TS = 4096  # T super-tile (per-DMA free size)
NSUP = T // TS

C_CHUNKS = [(0, 128), (128, 128), (256, C - 256)]  # contraction over channels
K_CHUNKS = [(0, 128), (128, 128), (256, D - 256)]  # emb dim (fourier features)
O_CHUNKS = [(0, 128), (128, 128), (256, O - 256)]  # output-channel chunks

WARM_MM = 12  # full-array dummy matmuls to trip the HAM gate to 8/8

_EXP = mybir.ActivationFunctionType.Exp


def _build_module() -> bass.Bass:
    nc = bacc.Bacc()
    meg_h = nc.dram_tensor("meg", [BPC, C, T], F16, kind="ExternalInput")
    embT_h = nc.dram_tensor("embT", [BPC, D, C], F16, kind="ExternalInput")
    headsTp_h = nc.dram_tensor("headsTp", [D, O], F16, kind="ExternalInput")
    out_h = nc.dram_tensor("out", [BPC, O, T], F16, kind="ExternalOutput")

    with tile.TileContext(nc) as tc:
        with (
            tc.tile_pool(name="const", bufs=1) as const,
            tc.tile_pool(name="small", bufs=2) as small,
            tc.tile_pool(name="megp", bufs=3) as megp,
            tc.tile_pool(name="outp", bufs=2) as outp,
            tc.tile_pool(name="psum", bufs=2, space="PSUM") as psum,
        ):
            # ---- persistent constants ----
            hT = []
            for ki, (k0, ksz) in enumerate(K_CHUNKS):
                t_ = const.tile([ksz, O], F16, tag=f"hT{ki}", name=f"hT{ki}")
                nc.sync.dma_start(out=t_, in_=headsTp_h[k0 : k0 + ksz, :])
                hT.append(t_)
            ones128 = const.tile([128, 128], F16, tag="ones", name="ones128")
            nc.vector.memset(ones128, 1.0)
            warm_src = const.tile([128, 512], F16, tag="warm", name="warm_src")
            nc.vector.memset(warm_src, 0.0)

            def ps_tile():
                # one 4-bank rotating slot; all psum users share the tag
                return psum.tile([128, 2048], F32, tag="ps", name="ps")

            # ---- PE warmup: full-array 512-col streams; HAM needs real
            # array activity, not just instruction busy ----
            warm_ps = ps_tile()
            for _ in range(WARM_MM):
                nc.tensor.matmul(
                    warm_ps[:, 0:512], ones128, warm_src, start=True, stop=True
                )

            # ---- phase 1: softmax weights for all batches ----
            def emit_softmax(b):
                embs = []
                for ki, (k0, ksz) in enumerate(K_CHUNKS):
                    e_ = small.tile(
                        [128, C], F16, tag=f"emb{ki}", name=f"emb{ki}", bufs=2
                    )[:ksz]
                    nc.sync.dma_start(out=e_, in_=embT_h[b, k0 : k0 + ksz, :])
                    embs.append(e_)
                expT = []
                for ci, (c0, csz) in enumerate(C_CHUNKS):
                    sc = ps_tile()[:csz, 0:O]
                    for ki in range(3):
                        nc.tensor.matmul(
                            sc,
                            embs[ki][:, c0 : c0 + csz],
                            hT[ki],
                            start=(ki == 0),
                            stop=(ki == 2),
                        )
                    e_ = small.tile(
                        [128, O], F16, tag=f"expT{ci}", name=f"expT{ci}", bufs=2
                    )[:csz]
                    nc.scalar.activation(e_, sc, _EXP)
                    expT.append(e_)
                # partition-replicated softmax sums via all-ones stationary
                sums = ps_tile()[:, 0:O]
                for ci, (c0, csz) in enumerate(C_CHUNKS):
                    nc.tensor.matmul(
                        sums,
                        ones128[:csz, :],
                        expT[ci],
                        start=(ci == 0),
                        stop=(ci == 2),
                    )
                inv = small.tile([128, O], F32, tag="inv", name="inv", bufs=2)
                nc.vector.reciprocal(inv, sums)
                wp = []
                for ci, (c0, csz) in enumerate(C_CHUNKS):
                    w_ = small.tile(
                        [128, O], F16, tag=f"wp{ci}", name=f"wp{ci}", bufs=2
                    )[:csz]
                    nc.vector.tensor_mul(w_, expT[ci], inv[:csz])
                    wp.append(w_)
                return wp

            wps = [emit_softmax(b) for b in range(BPC)]

            # ---- phase 2: PV, one dense 512-col PE stream ----
            for b in range(BPC):
                wp = wps[b]
                for ts in range(NSUP):
                    t0 = ts * TS
                    megs = []
                    for ci, (c0, csz) in enumerate(C_CHUNKS):
                        m_ = megp.tile([csz, TS], F16, tag=f"meg{ci}", name=f"meg{ci}")
                        nc.sync.dma_start(
                            out=m_, in_=meg_h[b, c0 : c0 + csz, t0 : t0 + TS]
                        )
                        megs.append(m_)
                    ostages = []
                    for oi, (o0, osz) in enumerate(O_CHUNKS):
                        ostages.append(
                            outp.tile(
                                [128, TS], F16, tag=f"ost{oi}", name=f"ost{oi}"
                            )[:osz]
                        )
                    for h in range(TS // 2048):
                        h0 = h * 2048
                        for oi, (o0, osz) in enumerate(O_CHUNKS):
                            ps = ps_tile()[:osz]
                            for ci in range(3):
                                w_ = wp[ci][:, o0 : o0 + osz]
                                for sl in range(4):
                                    nc.tensor.matmul(
                                        ps[:, sl * 512 : (sl + 1) * 512],
                                        w_,
                                        megs[ci][:, h0 + sl * 512 : h0 + (sl + 1) * 512],
                                        start=(ci == 0),
                                        stop=(ci == 2),
                                    )
                            # evict both halves concurrently on V + S: slot
                            # releases in half the time, PE never waits
                            nc.vector.tensor_copy(
                                ostages[oi][:, h0 : h0 + 1024], ps[:, 0:1024]
                            )
                            nc.scalar.copy(
                                ostages[oi][:, h0 + 1024 : h0 + 2048], ps[:, 1024:2048]
                            )
                    for oi, (o0, osz) in enumerate(O_CHUNKS):
                        nc.scalar.dma_start(
                            out=out_h[b, o0 : o0 + osz, t0 : t0 + TS], in_=ostages[oi]
                        )
    nc.compile()
    return nc


_MODULE_CACHE: list = []


def _get_module() -> bass.Bass:
    if not _MODULE_CACHE:
        _MODULE_CACHE.append(_build_module())
    return _MODULE_CACHE[0]


def _host_prep(meg, positions, heads):
    """Shard + lay out inputs for the 8 cores."""
    freqs = (TWO_PI / (1.0 + 2.0 * MARGIN)) * np.arange(N_FREQ, dtype=np.float64)
    pos = positions.astype(np.float64) + MARGIN
    loc = (
        pos[..., 0][..., None, None] * freqs[:, None]
        + pos[..., 1][..., None, None] * freqs[None, :]
    ).reshape(B, C, N_FREQ * N_FREQ)
    embT = np.concatenate(
        [np.cos(loc), np.sin(loc)], axis=2
    ).transpose(0, 2, 1).astype(np.float16)

    headsTp = np.ascontiguousarray(heads.T).astype(np.float16)  # [288, 270]

    in_maps = []
    for k in range(N_CORES):
        sl = slice(k * BPC, (k + 1) * BPC)
        in_maps.append(
            {
                "meg": np.ascontiguousarray(meg[sl]).astype(np.float16),
                "embT": np.ascontiguousarray(embT[sl]),
                "headsTp": headsTp,
            }
        )
    return in_maps


LAST_RESULTS = None  # BassKernelResults of the most recent kernel() call


def kernel(meg: np.ndarray, positions: np.ndarray, heads: np.ndarray) -> np.ndarray:
    global LAST_RESULTS
    from concourse.bass_utils import run_bass_kernel_spmd

    nc = _get_module()
    in_maps = _host_prep(
        np.asarray(meg, dtype=np.float32),
        np.asarray(positions, dtype=np.float32),
        np.asarray(heads, dtype=np.float32),
    )
    res = run_bass_kernel_spmd(nc, in_maps, core_ids=list(range(N_CORES)))
    LAST_RESULTS = res
    out = np.concatenate([r["out"] for r in res.results], axis=0)
    return out.astype(np.float32)
